# revision 11
# baseline (speedup 1.0000x reference)
"""CenterDiversityLoss kernel for 8 trn2 NeuronCores.

Reference: mean over i<j pairs of ||c_i - c_j||^2 / d for centers
(16384, 128) f32. Algebraic identity:

    sum_{i,j} ||c_i - c_j||^2 = 2*N*sum_i |c_i|^2 - 2*||sum_i c_i||^2

so loss = (N * sumsq - ||colsum||^2) / (count * d), count = N*(N-1)/2.
One streaming pass over the data — no Gram matrix, no collectives.

Sharding: row-blocks of 2048 per core. Five DMA chunks stream in
(512,512,512,256,256 rows; each SBUF tile holds consecutive rows per
partition, i.e. one contiguous >=1KB descriptor per partition). The two
small tail chunks keep the post-last-byte critical path short. Per chunk:
  - column sums: one tiny PE matmul per 128-column row-group slice
    (lhsT = data slice, rhs = ones column, N=1) -> psum[:, idx]
  - sum of squares: chunks 0-2 square on ACT and column-reduce through
    the same tiny-matmul trick; chunk 3 squares on DVE (tensor_mul) so
    the ACT engine is free when the final chunk lands; chunk 4 uses the
    fused ACT accum_out row-reduction, writing the result tile directly.
Each chunk gets its own scrap buffer (sq pool bufs=nchunk) — slot reuse
otherwise adds a WAR dependency from earlier chunks' PE reads onto the
tail square. One DVE copy moves psum -> SBUF and a single small DMA
writes [128, 31] floats per core. Host finishes the O(1) combine in
float64.

Prologue trim: Bass.__init__ unconditionally memsets four const tensors
on the Pool engine and then runs a full all-engine barrier (~500 ns)
before any kernel instruction can issue. For this kernel the barrier's
only job is ordering those memsets against their readers, so a
build-time patch skips the init memsets AND the init barrier entirely;
the two live consts (f32-0.0 ACT bias, f32-1.0 matmul ones) are
re-emitted inside the TileContext where the dependency tracker orders
them properly, and a 1-element dummy Square anchors the lazily-inserted
ACT function-table load to a dep-free op (otherwise Tile attaches it to
the first chunk's DMA and stalls the ACT chain). First input DMA then
issues at ~75 ns instead of ~500 ns.
"""

import numpy as np

import concourse.bacc as bacc
import concourse.bass as bass
import concourse.mybir as mybir
import concourse.tile as tile
from concourse.bass_utils import run_bass_kernel_spmd

N = 16384
D = 128
NCORES = 8
ROWS = N // NCORES          # 2048 rows per core
P = 128

CHUNK_ROWS = (512, 512, 512, 256, 256)
SQ_MODES = ("mm", "mm", "mm", "dve", "accum")
NCHUNK = len(CHUNK_ROWS)
SLICES = [r // 128 for r in CHUNK_ROWS]     # 128-col row-group slices
NS = sum(SLICES)                            # 16 column-sum psum slices
NSQ = sum(s for s, m in zip(SLICES, SQ_MODES) if m != "accum")  # 14
PSW = NS + NSQ                              # 30 psum columns
OUT_W = PSW + 1                             # + accum column = 31

_NC = None
LAST_RESULT = None  # BassKernelResults of the most recent run (for test.py)


def _make_bacc():
    """Bacc() with the init const memsets and the init all-engine barrier
    skipped; the live consts are re-emitted later under Tile tracking."""
    orig_memset = bass.BassGpSimd.memset
    orig_barrier = bass.Bass.all_engine_barrier

    def memset(self, ap, constant):
        nm = getattr(ap, "name", "") or ""
        if nm.startswith("const-"):
            return None
        return orig_memset(self, ap, constant)

    bass.BassGpSimd.memset = memset
    bass.Bass.all_engine_barrier = lambda self, **kw: None
    try:
        return bacc.Bacc("TRN2", target_bir_lowering=False, debug=False)
    finally:
        bass.BassGpSimd.memset = orig_memset
        bass.Bass.all_engine_barrier = orig_barrier


def _build():
    f32 = mybir.dt.float32
    nc = _make_bacc()
    x = nc.dram_tensor("x", [ROWS, D], f32, kind="ExternalInput")
    out = nc.dram_tensor("out", [P, OUT_W], f32, kind="ExternalOutput")

    with tile.TileContext(nc) as tc:
        with (
            tc.tile_pool(name="io", bufs=NCHUNK) as io,
            tc.tile_pool(name="sq", bufs=NCHUNK) as sqp,
            tc.tile_pool(name="acc", bufs=1) as acc,
            tc.tile_pool(name="ps", bufs=1, space="PSUM") as ps,
        ):
            ones = nc.const_aps.aps[(f32, 1.0)]
            zero_c = nc.const_aps.aps[(f32, 0.0)]
            nc.vector.memset(ones, 1.0)
            nc.gpsimd.memset(zero_c, 0.0)
            warm = acc.tile([P, 1], f32)
            nc.scalar.activation(
                warm[:], zero_c, mybir.ActivationFunctionType.Square)
            psum = ps.tile([P, PSW], f32)
            res = acc.tile([P, OUT_W], f32)

            row0 = 0
            cidx = 0
            midx = NS
            for k in range(NCHUNK):
                rows_k = CHUNK_ROWS[k]
                free_k = rows_k * D // P
                t = io.tile([P, free_k], f32, tag="in")
                src = x[row0:row0 + rows_k, :].rearrange("(p r) c -> p (r c)", p=P)
                row0 += rows_k
                nc.sync.dma_start(t[:], src)
                s = sqp.tile([P, free_k], f32, tag="sqt")
                mode = SQ_MODES[k]
                if mode == "accum":
                    nc.scalar.activation(
                        s[:], t[:], mybir.ActivationFunctionType.Square,
                        accum_out=res[:, PSW:PSW + 1])
                elif mode == "dve":
                    nc.vector.tensor_mul(s[:], t[:], t[:])
                else:
                    nc.scalar.activation(
                        s[:], t[:], mybir.ActivationFunctionType.Square)
                for j in range(SLICES[k]):
                    nc.tensor.matmul(
                        psum[:, cidx:cidx + 1], t[:, j * 128:(j + 1) * 128],
                        ones, start=True, stop=True)
                    cidx += 1
                    if mode != "accum":
                        nc.tensor.matmul(
                            psum[:, midx:midx + 1], s[:, j * 128:(j + 1) * 128],
                            ones, start=True, stop=True)
                        midx += 1
            nc.vector.tensor_copy(res[:, :PSW], psum[:])
            nc.sync.dma_start(out[:], res[:])
    nc.compile()
    return nc


def _get_nc():
    global _NC
    if _NC is None:
        _NC = _build()
    return _NC


def kernel(centers: np.ndarray, _trace: bool = False) -> np.ndarray:
    global LAST_RESULT
    centers = np.ascontiguousarray(np.asarray(centers, dtype=np.float32))
    assert centers.shape == (N, D)
    shards = centers.reshape(NCORES, ROWS, D)
    nc = _get_nc()
    in_maps = [{"x": shards[i]} for i in range(NCORES)]
    LAST_RESULT = run_bass_kernel_spmd(
        nc, in_maps, list(range(NCORES)), trace=_trace
    )
    outs = np.stack(
        [LAST_RESULT.results[i]["out"] for i in range(NCORES)]
    ).astype(np.float64)                       # (8, 128, 31)
    colsum = outs[:, :, :NS].sum(axis=(0, 2))  # (128,)
    sumsq = outs[:, :, NS:].sum()
    total = N * sumsq - colsum @ colsum
    count = N * (N - 1) / 2.0
    return np.asarray(total / (count * D), dtype=np.float32)
